# revision 1
# baseline (speedup 1.0000x reference)
"""Neural ODE (explicit Euler, 20 steps) Trainium2 Bass kernel.

z_{s+1} = z_s + h * (tanh(z_s @ W1 + b1) @ W2 + b2),  z0: [8192, 512] f32.

Strategy: pure data parallel over 8 NeuronCores (1024 batch rows each).
On each core the state is kept feature-major (zT: [512 features, 1024 batch])
resident in SBUF for all 20 steps; weights are replicated and resident.
Matmuls run in float32r (TF32-like, ~1e-4 rel err, 4x faster than fp32 on
the PE); the fp32 master copy of z is updated each step from the fp32 PSUM
result, so rounding does not accumulate in the state.

Bias folding: z_s = u_s + s*h*b2, where u_s carries only the matmul updates.
tanh input bias becomes b1 + s*(W1^T (h*b2)) (host-precomputed per step);
the final +20*h*b2 correction is applied on-device before the output
transpose. With the given inputs b1 = b2 = 0 so this is exact regardless.
"""

import numpy as np

P = 128
D = 512
B_FULL = 8192
NCORES = 8
BSH = B_FULL // NCORES  # 1024 batch rows per core
NSTEPS = 20
FT = D // P             # 4 feature tiles
CB = 512                # batch columns per chunk
NCHUNK = BSH // CB      # 2 chunks

_CACHE = {}


def _build_nc():
    import concourse.bacc as bacc
    import concourse.mybir as mybir
    import concourse.tile as tile
    from concourse.masks import make_identity

    f32 = mybir.dt.float32
    f32r = mybir.dt.float32r
    Tanh = mybir.ActivationFunctionType.Tanh
    add = mybir.AluOpType.add

    nc = bacc.Bacc("TRN2", target_bir_lowering=False, debug=False)
    z_in = nc.dram_tensor("z", [BSH, D], f32, kind="ExternalInput")
    w1_in = nc.dram_tensor("w1", [D, D], f32, kind="ExternalInput")
    w2_in = nc.dram_tensor("w2", [D, D], f32, kind="ExternalInput")  # pre-scaled by h
    # biases[p, jt, s] = b1[jt*128+p] + s * (W1^T (h*b2))[jt*128+p]
    b_in = nc.dram_tensor("biases", [P, FT, NSTEPS], f32, kind="ExternalInput")
    # bfin[p, ft] = NSTEPS * h * b2[ft*128+p]
    bf_in = nc.dram_tensor("bfin", [P, FT], f32, kind="ExternalInput")
    z_out = nc.dram_tensor("zout", [BSH, D], f32, kind="ExternalOutput")

    with tile.TileContext(nc) as tc:
        with (
            tc.tile_pool(name="stage", bufs=1) as stage,
            tc.tile_pool(name="wpool", bufs=1) as wpool,
            tc.tile_pool(name="zpool", bufs=2) as zpool,
            tc.tile_pool(name="zrpool", bufs=2) as zrpool,
            tc.tile_pool(name="apool", bufs=8) as apool,
            tc.tile_pool(name="ps", bufs=8, space="PSUM") as ps,
        ):
            # ---- load weights / constants ----
            w1s = stage.tile([P, FT, D], f32, tag="w1s")
            w2s = stage.tile([P, FT, D], f32, tag="w2s")
            nc.sync.dma_start(w1s[:], w1_in.ap().rearrange("(kt p) j -> p kt j", p=P))
            nc.sync.dma_start(w2s[:], w2_in.ap().rearrange("(kt p) j -> p kt j", p=P))
            w1r = wpool.tile([P, FT, D], f32r, tag="w1r")
            w2r = wpool.tile([P, FT, D], f32r, tag="w2r")
            nc.vector.tensor_copy(w1r[:], w1s[:])
            nc.scalar.copy(w2r[:], w2s[:])
            bias_sb = wpool.tile([P, FT, NSTEPS], f32, tag="bias")
            nc.sync.dma_start(bias_sb[:], b_in.ap())
            bfin_sb = wpool.tile([P, FT], f32, tag="bfin")
            nc.sync.dma_start(bfin_sb[:], bf_in.ap())
            ident = wpool.tile([P, P], f32, tag="id")
            make_identity(nc, ident[:])

            # ---- load z and transpose to feature-major ----
            zb = stage.tile([P, BSH // P, D], f32, tag="zb")
            nc.sync.dma_start(zb[:], z_in.ap().rearrange("(bt p) d -> p bt d", p=P))

            z_t = {}   # fp32 master, [c][ft] -> [128, CB]
            zr_t = {}  # f32r copy for matmul rhs
            for c in range(NCHUNK):
                for ft in range(FT):
                    zm = zpool.tile([P, CB], f32, tag=f"z_{c}_{ft}")
                    zr = zrpool.tile([P, CB], f32r, tag=f"zr_{c}_{ft}")
                    for ct in range(CB // P):
                        tr = ps.tile([P, P], f32, tag="ps")
                        nc.tensor.transpose(
                            tr[:], zb[:, c * (CB // P) + ct, ft * P:(ft + 1) * P],
                            ident[:],
                        )
                        nc.vector.tensor_copy(zm[:, ct * P:(ct + 1) * P], tr[:])
                        nc.scalar.copy(zr[:, ct * P:(ct + 1) * P], tr[:])
                    z_t[(c, ft)] = zm
                    zr_t[(c, ft)] = zr

            # ---- 20 Euler steps ----
            for s in range(NSTEPS):
                for c in range(NCHUNK):
                    a_t = []
                    for jt in range(FT):
                        ph = ps.tile([P, CB], f32, tag="ps")
                        for kt in range(FT):
                            nc.tensor.matmul(
                                ph[:],
                                w1r[:, kt, jt * P:(jt + 1) * P],
                                zr_t[(c, kt)][:],
                                start=(kt == 0), stop=(kt == FT - 1),
                            )
                        a = apool.tile([P, CB], f32r, tag="a")
                        nc.scalar.activation(
                            a[:], ph[:], Tanh, bias=bias_sb[:, jt, s:s + 1], scale=1.0,
                        )
                        a_t.append(a)
                    for jt2 in range(FT):
                        py = ps.tile([P, CB], f32, tag="ps")
                        for jt in range(FT):
                            nc.tensor.matmul(
                                py[:],
                                w2r[:, jt, jt2 * P:(jt2 + 1) * P],
                                a_t[jt][:],
                                start=(jt == 0), stop=(jt == FT - 1),
                            )
                        z_old = z_t[(c, jt2)]
                        zm = zpool.tile([P, CB], f32, tag=f"z_{c}_{jt2}")
                        zr = zrpool.tile([P, CB], f32r, tag=f"zr_{c}_{jt2}")
                        nc.vector.tensor_add(zm[:], z_old[:], py[:])
                        nc.vector.tensor_add(zr[:], z_old[:], py[:])
                        z_t[(c, jt2)] = zm
                        zr_t[(c, jt2)] = zr

            # ---- final correction and transpose back ----
            zob = stage.tile([P, BSH // P, D], f32, tag="zob")
            for c in range(NCHUNK):
                for ft in range(FT):
                    zc = zpool.tile([P, CB], f32, tag=f"z_{c}_{ft}")
                    nc.vector.tensor_scalar(
                        zc[:], z_t[(c, ft)][:], bfin_sb[:, ft:ft + 1], None, add,
                    )
                    for ct in range(CB // P):
                        tr = ps.tile([P, P], f32, tag="ps")
                        nc.tensor.transpose(
                            tr[:], zc[:, ct * P:(ct + 1) * P], ident[:],
                        )
                        if ct % 2 == 0:
                            nc.vector.tensor_copy(
                                zob[:, c * (CB // P) + ct, ft * P:(ft + 1) * P], tr[:],
                            )
                        else:
                            nc.scalar.copy(
                                zob[:, c * (CB // P) + ct, ft * P:(ft + 1) * P], tr[:],
                            )
            nc.sync.dma_start(z_out.ap().rearrange("(bt p) d -> p bt d", p=P), zob[:])

    nc.finalize()
    return nc


def _get_nc():
    if "nc" not in _CACHE:
        _CACHE["nc"] = _build_nc()
    return _CACHE["nc"]


def _prepare_inputs(z0, t, W1, b1, W2, b2):
    z0 = np.ascontiguousarray(np.asarray(z0, dtype=np.float32))
    t = np.asarray(t, dtype=np.float32)
    W1 = np.ascontiguousarray(np.asarray(W1, dtype=np.float32))
    b1 = np.asarray(b1, dtype=np.float64)
    W2 = np.ascontiguousarray(np.asarray(W2, dtype=np.float32))
    b2 = np.asarray(b2, dtype=np.float64)

    h = (float(t[1]) - float(t[0])) / NSTEPS
    W2h = np.ascontiguousarray((W2.astype(np.float64) * h).astype(np.float32))
    b2h = b2 * h
    wtb = W1.astype(np.float64).T @ b2h  # [D]
    biases = np.stack(
        [b1 + s * wtb for s in range(NSTEPS)], axis=0
    ).astype(np.float32)  # [NSTEPS, D]
    biases_tiled = np.ascontiguousarray(
        biases.reshape(NSTEPS, FT, P).transpose(2, 1, 0)
    )  # [P, FT, NSTEPS]
    bfin_tiled = np.ascontiguousarray(
        (NSTEPS * b2h).astype(np.float32).reshape(FT, P).T
    )  # [P, FT]

    in_maps = []
    for i in range(NCORES):
        in_maps.append({
            "z": z0[i * BSH:(i + 1) * BSH],
            "w1": W1,
            "w2": W2h,
            "biases": biases_tiled,
            "bfin": bfin_tiled,
        })
    return in_maps


def _run(in_maps, trace=False):
    from concourse import bass_utils

    nc = _get_nc()
    res = bass_utils.run_bass_kernel_spmd(
        nc, in_maps, core_ids=list(range(NCORES)), trace=trace,
    )
    return res


def kernel(z0, t, W1, b1, W2, b2):
    in_maps = _prepare_inputs(z0, t, W1, b1, W2, b2)
    res = _run(in_maps)
    out = np.concatenate([r["zout"] for r in res.results], axis=0)
    return out.astype(np.float32)


# revision 3
# speedup vs baseline: 1.0214x; 1.0214x over previous
"""Neural ODE (explicit Euler, 20 steps) Trainium2 Bass kernel.

z_{s+1} = z_s + h * (tanh(z_s @ W1 + b1) @ W2 + b2),  z0: [8192, 512] f32.

Strategy: pure data parallel over 8 NeuronCores (1024 batch rows each).
On each core the state is kept feature-major (zT: [512 features, 1024 batch])
resident in SBUF for all 20 steps; weights are replicated and resident.
Matmuls run in float32r (TF32-like, ~1e-4 rel err, 4x faster than fp32 on
the PE); the fp32 master copy of z is updated each step from the fp32 PSUM
result, so rounding does not accumulate in the state.

Bias folding: z_s = u_s + s*h*b2, where u_s carries only the matmul updates.
tanh input bias becomes b1 + s*(W1^T (h*b2)) (host-precomputed per step);
the final +20*h*b2 correction is applied on-device before the output
transpose. With the given inputs b1 = b2 = 0 so this is exact regardless.
"""

import numpy as np

P = 128
D = 512
B_FULL = 8192
NCORES = 8
BSH = B_FULL // NCORES  # 1024 batch rows per core
NSTEPS = 20
FT = D // P             # 4 feature tiles
CB = 512                # batch columns per chunk
NCHUNK = BSH // CB      # 2 chunks
CT = CB // P            # 4 batch col-tiles per chunk

_CACHE = {}


def _build_nc():
    import concourse.bacc as bacc
    import concourse.mybir as mybir
    import concourse.tile as tile
    from concourse.masks import make_identity

    f32 = mybir.dt.float32
    f32r = mybir.dt.float32r
    Tanh = mybir.ActivationFunctionType.Tanh
    add = mybir.AluOpType.add

    nc = bacc.Bacc("TRN2", target_bir_lowering=False, debug=False)
    z_in = nc.dram_tensor("z", [BSH, D], f32, kind="ExternalInput")
    w1_in = nc.dram_tensor("w1", [D, D], f32, kind="ExternalInput")
    w2_in = nc.dram_tensor("w2", [D, D], f32, kind="ExternalInput")  # pre-scaled by h
    # biases[p, jt, s] = b1[jt*128+p] + s * (W1^T (h*b2))[jt*128+p]
    b_in = nc.dram_tensor("biases", [P, FT, NSTEPS], f32, kind="ExternalInput")
    # bfin[p, ft] = NSTEPS * h * b2[ft*128+p]
    bf_in = nc.dram_tensor("bfin", [P, FT], f32, kind="ExternalInput")
    z_out = nc.dram_tensor("zout", [BSH, D], f32, kind="ExternalOutput")

    z_in_t = z_in.ap().rearrange("(bt p) d -> p bt d", p=P)
    z_out_t = z_out.ap().rearrange("(bt p) d -> p bt d", p=P)

    with tile.TileContext(nc) as tc:
        with (
            tc.tile_pool(name="stage", bufs=1) as stage,
            tc.tile_pool(name="wpool", bufs=1) as wpool,
            tc.tile_pool(name="zpool", bufs=2) as zpool,
            tc.tile_pool(name="zrpool", bufs=2) as zrpool,
            tc.tile_pool(name="apool", bufs=8) as apool,
            tc.tile_pool(name="opool", bufs=2) as opool,
            tc.tile_pool(name="ps", bufs=8, space="PSUM") as ps,
        ):
            # ---- z input DMAs first (gate the start of the step pipeline) ----
            zb = {}
            for bt in range(BSH // P):
                zb[bt] = stage.tile([P, D], f32, tag=f"zb{bt}", name=f"zb{bt}")
                nc.sync.dma_start(zb[bt][:], z_in_t[:, bt, :])

            # ---- weights / constants ----
            w1s = stage.tile([P, FT, D], f32, tag="w1s")
            nc.sync.dma_start(w1s[:], w1_in.ap().rearrange("(kt p) j -> p kt j", p=P))
            w1r = wpool.tile([P, FT, D], f32r, tag="w1r")
            nc.vector.tensor_copy(w1r[:], w1s[:])
            w2s = stage.tile([P, FT, D], f32, tag="w2s")
            nc.sync.dma_start(w2s[:], w2_in.ap().rearrange("(kt p) j -> p kt j", p=P))
            w2r = wpool.tile([P, FT, D], f32r, tag="w2r")
            nc.vector.tensor_copy(w2r[:], w2s[:])
            bias_sb = wpool.tile([P, FT, NSTEPS], f32, tag="bias")
            nc.sync.dma_start(bias_sb[:], b_in.ap())
            bfin_sb = wpool.tile([P, FT], f32, tag="bfin")
            nc.sync.dma_start(bfin_sb[:], bf_in.ap())
            ident = wpool.tile([P, P], f32, tag="id")
            make_identity(nc, ident[:])

            # ---- transpose z to feature-major ----
            z_t = {}   # fp32 master, (c, ft) -> [128, CB]
            zr_t = {}  # f32r copy for matmul rhs
            for c in range(NCHUNK):
                for ft in range(FT):
                    zm = zpool.tile([P, CB], f32, tag=f"z_{c}_{ft}")
                    for ct in range(CT):
                        tr = ps.tile([P, P], f32, tag="ps")
                        nc.tensor.transpose(
                            tr[:], zb[c * CT + ct][:, ft * P:(ft + 1) * P], ident[:],
                        )
                        nc.vector.tensor_copy(zm[:, ct * P:(ct + 1) * P], tr[:])
                    zr = zrpool.tile([P, CB], f32r, tag=f"zr_{c}_{ft}")
                    nc.scalar.copy(zr[:], zm[:])
                    z_t[(c, ft)] = zm
                    zr_t[(c, ft)] = zr

            # ---- 20 Euler steps ----
            for s in range(NSTEPS):
                last = s == NSTEPS - 1
                for c in range(NCHUNK):
                    a_t = []
                    for jt in range(FT):
                        ph = ps.tile([P, CB], f32, tag="ps")
                        for kt in range(FT):
                            nc.tensor.matmul(
                                ph[:],
                                w1r[:, kt, jt * P:(jt + 1) * P],
                                zr_t[(c, kt)][:],
                                start=(kt == 0), stop=(kt == FT - 1),
                            )
                        a = apool.tile([P, CB], f32r, tag="a")
                        nc.scalar.activation(
                            a[:], ph[:], Tanh, bias=bias_sb[:, jt, s:s + 1], scale=1.0,
                        )
                        a_t.append(a)
                    for jt2 in range(FT):
                        py = ps.tile([P, CB], f32, tag="ps")
                        for jt in range(FT):
                            nc.tensor.matmul(
                                py[:],
                                w2r[:, jt, jt2 * P:(jt2 + 1) * P],
                                a_t[jt][:],
                                start=(jt == 0), stop=(jt == FT - 1),
                            )
                        z_old = z_t[(c, jt2)]
                        zm = zpool.tile([P, CB], f32, tag=f"z_{c}_{jt2}")
                        nc.vector.tensor_add(zm[:], z_old[:], py[:])
                        z_t[(c, jt2)] = zm
                        if not last:
                            zr = zrpool.tile([P, CB], f32r, tag=f"zr_{c}_{jt2}")
                            nc.vector.tensor_add(zr[:], z_old[:], py[:])
                            zr_t[(c, jt2)] = zr

            # ---- final correction, transpose back, store ----
            for c in range(NCHUNK):
                for ft in range(FT):
                    zc = zpool.tile([P, CB], f32, tag=f"z_{c}_{ft}")
                    nc.vector.tensor_scalar(
                        zc[:], z_t[(c, ft)][:], bfin_sb[:, ft:ft + 1], None, add,
                    )
                    zo = opool.tile([P, CT, P], f32, tag="zo")
                    for ct in range(CT):
                        tr = ps.tile([P, P], f32, tag="ps")
                        nc.tensor.transpose(
                            tr[:], zc[:, ct * P:(ct + 1) * P], ident[:],
                        )
                        if ct % 2 == 0:
                            nc.vector.tensor_copy(zo[:, ct, :], tr[:])
                        else:
                            nc.scalar.copy(zo[:, ct, :], tr[:])
                    nc.sync.dma_start(
                        z_out_t[:, c * CT:(c + 1) * CT, ft * P:(ft + 1) * P], zo[:],
                    )

    nc.finalize()
    return nc


def _get_nc():
    if "nc" not in _CACHE:
        _CACHE["nc"] = _build_nc()
    return _CACHE["nc"]


def _prepare_inputs(z0, t, W1, b1, W2, b2):
    z0 = np.ascontiguousarray(np.asarray(z0, dtype=np.float32))
    t = np.asarray(t, dtype=np.float32)
    W1 = np.ascontiguousarray(np.asarray(W1, dtype=np.float32))
    b1 = np.asarray(b1, dtype=np.float64)
    W2 = np.ascontiguousarray(np.asarray(W2, dtype=np.float32))
    b2 = np.asarray(b2, dtype=np.float64)

    h = (float(t[1]) - float(t[0])) / NSTEPS
    W2h = np.ascontiguousarray((W2.astype(np.float64) * h).astype(np.float32))
    b2h = b2 * h
    wtb = W1.astype(np.float64).T @ b2h  # [D]
    biases = np.stack(
        [b1 + s * wtb for s in range(NSTEPS)], axis=0
    ).astype(np.float32)  # [NSTEPS, D]
    biases_tiled = np.ascontiguousarray(
        biases.reshape(NSTEPS, FT, P).transpose(2, 1, 0)
    )  # [P, FT, NSTEPS]
    bfin_tiled = np.ascontiguousarray(
        (NSTEPS * b2h).astype(np.float32).reshape(FT, P).T
    )  # [P, FT]

    in_maps = []
    for i in range(NCORES):
        in_maps.append({
            "z": z0[i * BSH:(i + 1) * BSH],
            "w1": W1,
            "w2": W2h,
            "biases": biases_tiled,
            "bfin": bfin_tiled,
        })
    return in_maps


def _run(in_maps, trace=False):
    from concourse import bass_utils

    nc = _get_nc()
    res = bass_utils.run_bass_kernel_spmd(
        nc, in_maps, core_ids=list(range(NCORES)), trace=trace,
    )
    return res


def kernel(z0, t, W1, b1, W2, b2):
    in_maps = _prepare_inputs(z0, t, W1, b1, W2, b2)
    res = _run(in_maps)
    out = np.concatenate([r["zout"] for r in res.results], axis=0)
    return out.astype(np.float32)


# revision 4
# speedup vs baseline: 1.0995x; 1.0765x over previous
"""Neural ODE (explicit Euler, 20 steps) Trainium2 Bass kernel.

z_{s+1} = z_s + h * (tanh(z_s @ W1 + b1) @ W2 + b2),  z0: [8192, 512] f32.

Strategy: pure data parallel over 8 NeuronCores (1024 batch rows each).
On each core the state is kept feature-major (zT: [512 features, 1024 batch])
resident in SBUF for all 20 steps; weights are replicated and resident.
Matmuls run in float32r (TF32-like, ~1e-4 rel err, 4x faster than fp32 on
the PE); the fp32 master copy of z is updated each step from the fp32 PSUM
result, so rounding does not accumulate in the state.

Bias folding: z_s = u_s + s*h*b2, where u_s carries only the matmul updates.
tanh input bias becomes b1 + s*(W1^T (h*b2)) (host-precomputed per step);
the final +20*h*b2 correction is applied on-device before the output
transpose. With the given inputs b1 = b2 = 0 so this is exact regardless.
"""

import numpy as np

P = 128
D = 512
B_FULL = 8192
NCORES = 8
BSH = B_FULL // NCORES  # 1024 batch rows per core
NSTEPS = 20
FT = D // P             # 4 feature tiles
CB = 512                # batch columns per chunk
NCHUNK = BSH // CB      # 2 chunks
CT = CB // P            # 4 batch col-tiles per chunk

_CACHE = {}


def _build_nc():
    import concourse.bacc as bacc
    import concourse.mybir as mybir
    import concourse.tile as tile
    from concourse.masks import make_identity

    import os
    f32 = mybir.dt.float32
    f32r = {"f32r": mybir.dt.float32r, "f16": mybir.dt.float16,
            "bf16": mybir.dt.bfloat16}[os.environ.get("MM_DTYPE", "f32r")]
    Tanh = mybir.ActivationFunctionType.Tanh
    add = mybir.AluOpType.add

    nc = bacc.Bacc("TRN2", target_bir_lowering=False, debug=False)
    z_in = nc.dram_tensor("z", [BSH, D], f32, kind="ExternalInput")
    w1_in = nc.dram_tensor("w1", [D, D], f32, kind="ExternalInput")
    w2_in = nc.dram_tensor("w2", [D, D], f32, kind="ExternalInput")  # pre-scaled by h
    # biases[p, jt, s] = b1[jt*128+p] + s * (W1^T (h*b2))[jt*128+p]
    b_in = nc.dram_tensor("biases", [P, FT, NSTEPS], f32, kind="ExternalInput")
    # bfin[p, ft] = NSTEPS * h * b2[ft*128+p]
    bf_in = nc.dram_tensor("bfin", [P, FT], f32, kind="ExternalInput")
    z_out = nc.dram_tensor("zout", [BSH, D], f32, kind="ExternalOutput")

    z_in_t = z_in.ap().rearrange("(bt p) d -> p bt d", p=P)
    z_out_t = z_out.ap().rearrange("(bt p) d -> p bt d", p=P)

    with tile.TileContext(nc) as tc:
        with (
            tc.tile_pool(name="stage", bufs=1) as stage,
            tc.tile_pool(name="wpool", bufs=1) as wpool,
            tc.tile_pool(name="zpool", bufs=2) as zpool,
            tc.tile_pool(name="zrpool", bufs=2) as zrpool,
            tc.tile_pool(name="apool", bufs=8) as apool,
            tc.tile_pool(name="opool", bufs=2) as opool,
            tc.tile_pool(name="ps", bufs=8, space="PSUM") as ps,
        ):
            # ---- z input DMAs first (gate the start of the step pipeline) ----
            zb = {}
            for bt in range(BSH // P):
                zb[bt] = stage.tile([P, D], f32, tag=f"zb{bt}", name=f"zb{bt}")
                nc.sync.dma_start(zb[bt][:], z_in_t[:, bt, :])

            # ---- weights / constants ----
            w1s = stage.tile([P, FT, D], f32, tag="w1s")
            nc.sync.dma_start(w1s[:], w1_in.ap().rearrange("(kt p) j -> p kt j", p=P))
            w1r = wpool.tile([P, FT, D], f32r, tag="w1r")
            nc.vector.tensor_copy(w1r[:], w1s[:])
            w2s = stage.tile([P, FT, D], f32, tag="w2s")
            nc.sync.dma_start(w2s[:], w2_in.ap().rearrange("(kt p) j -> p kt j", p=P))
            w2r = wpool.tile([P, FT, D], f32r, tag="w2r")
            nc.vector.tensor_copy(w2r[:], w2s[:])
            bias_sb = wpool.tile([P, FT, NSTEPS], f32, tag="bias")
            nc.sync.dma_start(bias_sb[:], b_in.ap())
            bfin_sb = wpool.tile([P, FT], f32, tag="bfin")
            nc.sync.dma_start(bfin_sb[:], bf_in.ap())
            ident = wpool.tile([P, P], f32, tag="id")
            make_identity(nc, ident[:])

            # ---- transpose z to feature-major ----
            z_t = {}   # fp32 master, (c, ft) -> [128, CB]
            zr_t = {}  # f32r copy for matmul rhs
            for c in range(NCHUNK):
                for ft in range(FT):
                    zm = zpool.tile([P, CB], f32, tag=f"z_{c}_{ft}")
                    for ct in range(CT):
                        tr = ps.tile([P, P], f32, tag="ps")
                        nc.tensor.transpose(
                            tr[:], zb[c * CT + ct][:, ft * P:(ft + 1) * P], ident[:],
                        )
                        nc.vector.tensor_copy(zm[:, ct * P:(ct + 1) * P], tr[:])
                    zr = zrpool.tile([P, CB], f32r, tag=f"zr_{c}_{ft}")
                    nc.scalar.copy(zr[:], zm[:])
                    z_t[(c, ft)] = zm
                    zr_t[(c, ft)] = zr

            # ---- 20 Euler steps ----
            for s in range(NSTEPS):
                last = s == NSTEPS - 1
                for c in range(NCHUNK):
                    a_t = []
                    for jt in range(FT):
                        ph = ps.tile([P, CB], f32, tag="ps")
                        for kt in range(FT):
                            nc.tensor.matmul(
                                ph[:],
                                w1r[:, kt, jt * P:(jt + 1) * P],
                                zr_t[(c, kt)][:],
                                start=(kt == 0), stop=(kt == FT - 1),
                            )
                        a = apool.tile([P, CB], f32r, tag="a")
                        nc.scalar.activation(
                            a[:], ph[:], Tanh, bias=bias_sb[:, jt, s:s + 1], scale=1.0,
                        )
                        a_t.append(a)
                    for jt2 in range(FT):
                        py = ps.tile([P, CB], f32, tag="ps")
                        for jt in range(FT):
                            nc.tensor.matmul(
                                py[:],
                                w2r[:, jt, jt2 * P:(jt2 + 1) * P],
                                a_t[jt][:],
                                start=(jt == 0), stop=(jt == FT - 1),
                            )
                        z_old = z_t[(c, jt2)]
                        zm = zpool.tile([P, CB], f32, tag=f"z_{c}_{jt2}")
                        nc.vector.tensor_add(zm[:], z_old[:], py[:])
                        z_t[(c, jt2)] = zm
                        if not last:
                            zr = zrpool.tile([P, CB], f32r, tag=f"zr_{c}_{jt2}")
                            nc.vector.tensor_add(zr[:], z_old[:], py[:])
                            zr_t[(c, jt2)] = zr

            # ---- final correction, transpose back, store ----
            for c in range(NCHUNK):
                for ft in range(FT):
                    zc = zpool.tile([P, CB], f32, tag=f"z_{c}_{ft}")
                    nc.vector.tensor_scalar(
                        zc[:], z_t[(c, ft)][:], bfin_sb[:, ft:ft + 1], None, add,
                    )
                    zo = opool.tile([P, CT, P], f32, tag="zo")
                    for ct in range(CT):
                        tr = ps.tile([P, P], f32, tag="ps")
                        nc.tensor.transpose(
                            tr[:], zc[:, ct * P:(ct + 1) * P], ident[:],
                        )
                        if ct % 2 == 0:
                            nc.vector.tensor_copy(zo[:, ct, :], tr[:])
                        else:
                            nc.scalar.copy(zo[:, ct, :], tr[:])
                    nc.sync.dma_start(
                        z_out_t[:, c * CT:(c + 1) * CT, ft * P:(ft + 1) * P], zo[:],
                    )

    nc.finalize()
    return nc


def _get_nc():
    if "nc" not in _CACHE:
        _CACHE["nc"] = _build_nc()
    return _CACHE["nc"]


def _prepare_inputs(z0, t, W1, b1, W2, b2):
    z0 = np.ascontiguousarray(np.asarray(z0, dtype=np.float32))
    t = np.asarray(t, dtype=np.float32)
    W1 = np.ascontiguousarray(np.asarray(W1, dtype=np.float32))
    b1 = np.asarray(b1, dtype=np.float64)
    W2 = np.ascontiguousarray(np.asarray(W2, dtype=np.float32))
    b2 = np.asarray(b2, dtype=np.float64)

    h = (float(t[1]) - float(t[0])) / NSTEPS
    W2h = np.ascontiguousarray((W2.astype(np.float64) * h).astype(np.float32))
    b2h = b2 * h
    wtb = W1.astype(np.float64).T @ b2h  # [D]
    biases = np.stack(
        [b1 + s * wtb for s in range(NSTEPS)], axis=0
    ).astype(np.float32)  # [NSTEPS, D]
    biases_tiled = np.ascontiguousarray(
        biases.reshape(NSTEPS, FT, P).transpose(2, 1, 0)
    )  # [P, FT, NSTEPS]
    bfin_tiled = np.ascontiguousarray(
        (NSTEPS * b2h).astype(np.float32).reshape(FT, P).T
    )  # [P, FT]

    in_maps = []
    for i in range(NCORES):
        in_maps.append({
            "z": z0[i * BSH:(i + 1) * BSH],
            "w1": W1,
            "w2": W2h,
            "biases": biases_tiled,
            "bfin": bfin_tiled,
        })
    return in_maps


def _run(in_maps, trace=False):
    from concourse import bass_utils

    nc = _get_nc()
    res = bass_utils.run_bass_kernel_spmd(
        nc, in_maps, core_ids=list(range(NCORES)), trace=trace,
    )
    return res


def kernel(z0, t, W1, b1, W2, b2):
    in_maps = _prepare_inputs(z0, t, W1, b1, W2, b2)
    res = _run(in_maps)
    out = np.concatenate([r["zout"] for r in res.results], axis=0)
    return out.astype(np.float32)
